# revision 6
# baseline (speedup 1.0000x reference)
"""Trainium2 Bass kernel for nn_Encoder_79843442033106 (retrieval_knn).

Reference computation:
  queries xq[b,k,:] (1024 x 2016, fp16 values) are matched against a codebook
  c (16001 x 2016) under squared L2 distance, searching the concatenation
  [d0, d1, d1, d0] where d0 = ||x-c||^2 and d1 = ||x-(1-c)||^2; the argmin
  index is emitted LSB-first as 32 bits -> output [64, 512] int32.

Strategy (quantized screen + exact rescore, codebook axis M sharded 8 ways):
  * Device: one fp8e4 DoubleRow GEMM per core computes a screening score
      psum[q,m] ~= 2<x,c> - ||c||^2  (= x2 - d0, up to fp8 quantization)
    with K padded 2016->2048 (8 double-row contraction steps of 256) and
    -||c||^2 folded into 5 of the padding rows as a greedy fp8 decomposition
    (query-side coefficient 1).  fp8 DoubleRow streams 256 contraction rows
    per column-cycle -- 2x the fp16 matmul rate, measured ~61us/core for the
    1024x2048x2048 product vs ~133us for the exact fp16 equivalent.
  * DVE (fp16, 2x element rate): v = -t2 - (c2+cn2);  h = (v + 2xs) max t2
    so h[m] ~= max(x2-d0, x2-d1); then per 1024-wide half-chunk the DVE
    max/max_index ops emit the TOP-8 values and indices per query.
  * Host: 8 cores x 2 halves x top8 = 128 candidates/query; rescore them
    exactly in f64 with the reference's tie-breaking ((d, d0-before-d1,
    lowest m)); emit the argmin's 32 bits.  Screening noise is ~1.0 (std)
    while the true argmin ranks at worst 2nd within its half-chunk on this
    distribution -- top-8 gives 4x headroom (verified 0/1024 mismatches).
"""

import numpy as np

import concourse.bass as bass
import concourse.tile as tile
from concourse import bacc, mybir
from concourse.bass_utils import run_bass_kernel_spmd

# Problem constants (hardcoded per the harness contract).
B = 64
KSLOT = 16
D = 2016
M = 16001
NBITS = 32
BK = B * KSLOT           # 1024 queries
NCORES = 8
MLOC = 2048              # per-core codebook buffer columns (bank-aligned)
MREAL = 2001             # streamed columns per core (8*2001 >= 16001)
KC = 16                  # 128-row contraction chunks (16*128 = 2048 >= D+5)
NQT = BK // 128          # 8 query tiles
NAUG = 5                 # fp8 rows folding -c2 (greedy, cap 240)
PAD_ROW = np.float64(240.0)  # per-aug-row magnitude for padded columns
# PSUM-bank-aligned m-chunks: starts on 512-f32 boundaries, last one short.
CHUNKS = ((0, 512), (512, 1024), (1024, 1536), (1536, MREAL))
HALVES = ((0, 1024), (1024, MREAL))

_compiled = {}


def _build_program(repeat: int = 1) -> bass.Bass:
    """repeat>1 replays the whole compute body N times inside one NEFF --
    used only by the bench to measure per-iteration device time
    differentially (dispatch overhead cancels)."""
    f16 = mybir.dt.float16
    f32 = mybir.dt.float32
    f8 = mybir.dt.float8e4
    u32 = mybir.dt.uint32

    nc = bacc.Bacc("TRN2", debug=False, num_devices=NCORES)

    xq8 = nc.dram_tensor("xq8", [128, KC, BK], f8, kind="ExternalInput").ap()
    ct8 = nc.dram_tensor("ct8", [128, KC, MLOC], f8, kind="ExternalInput").ap()
    srep = nc.dram_tensor("srep", [128, MLOC], f16, kind="ExternalInput").ap()
    xs2 = nc.dram_tensor("xs2", [128, NQT], f16, kind="ExternalInput").ap()
    outv = nc.dram_tensor("outv", [BK, 2, 8], f16, kind="ExternalOutput").ap()
    outi = nc.dram_tensor("outi", [BK, 2, 8], u32, kind="ExternalOutput").ap()

    with tile.TileContext(nc) as tc:
        with (
            tc.tile_pool(name="const", bufs=1) as const_pool,
            tc.tile_pool(name="psum", bufs=2, space="PSUM") as psum_pool,
            tc.tile_pool(name="work", bufs=3) as work_pool,
            tc.tile_pool(name="outs", bufs=3) as out_pool,
        ):
            xs2_t = const_pool.tile([128, NQT], f16, tag="xs2")
            srep_t = const_pool.tile([128, MLOC], f16, tag="srep")
            xt = const_pool.tile([128, KC, BK], f8, tag="xq8")
            ctt = const_pool.tile([128, KC, MLOC], f8, tag="ct8")
            # Queries first (small), then the codes the first matmuls touch.
            nc.sync.dma_start(xt[:], xq8[:, :, :])
            nc.sync.dma_start(xs2_t[:], xs2[:, :])
            nc.sync.dma_start(ctt[:], ct8[:, :, :])
            nc.sync.dma_start(srep_t[:], srep[:, :])

            for rep in range(repeat):
                for qt in range(NQT):
                    # GEMM: psum[q, m] = 2<x,c> - c2 (fp8 DoubleRow, f32 acc).
                    # Chunks start on PSUM bank boundaries (512 f32): a
                    # matmul output may not straddle banks.
                    ps = psum_pool.tile([128, MLOC], f32, tag="ps")
                    for j in range(KC // 2):
                        for lo, hi in CHUNKS:
                            nc.tensor.matmul(
                                ps[:, lo:hi],
                                lhsT=xt[:, 2 * j:2 * j + 2,
                                        qt * 128:(qt + 1) * 128],
                                rhs=ctt[:, 2 * j:2 * j + 2, lo:hi],
                                start=(j == 0),
                                stop=(j == KC // 2 - 1),
                                perf_mode=mybir.MatmulPerfMode.DoubleRow,
                            )

                    # ACT stages PSUM->SBUF as fp16 (frees the PSUM banks and
                    # gives the DVE its 2x 16-bit element rate); per-chunk so
                    # the copy starts as soon as each bank's accumulation ends.
                    t2 = work_pool.tile([128, MLOC], f16, tag="t2")
                    for lo, hi in CHUNKS:
                        nc.scalar.copy(t2[:, lo:hi], ps[:, lo:hi])
                    v = work_pool.tile([128, MLOC], f16, tag="v")
                    h = work_pool.tile([128, MLOC], f16, tag="h")
                    for half, (hlo, hhi) in enumerate(HALVES):
                        sl = slice(hlo, hhi)
                        nc.vector.scalar_tensor_tensor(
                            v[:, sl], in0=t2[:, sl], scalar=-1.0,
                            in1=srep_t[:, sl],
                            op0=mybir.AluOpType.mult,
                            op1=mybir.AluOpType.subtract,
                        )
                        nc.vector.scalar_tensor_tensor(
                            h[:, sl], in0=v[:, sl],
                            scalar=xs2_t[:, qt:qt + 1], in1=t2[:, sl],
                            op0=mybir.AluOpType.add, op1=mybir.AluOpType.max,
                        )
                        v_t = out_pool.tile([128, 8], f16, tag="v8")
                        nc.vector.max(v_t[:], h[:, sl])
                        i_t = out_pool.tile([128, 8], u32, tag="i8")
                        nc.vector.max_index(i_t[:], v_t[:], h[:, sl])
                        nc.sync.dma_start(
                            outv[qt * 128:(qt + 1) * 128, half, :], v_t[:]
                        )
                        nc.sync.dma_start(
                            outi[qt * 128:(qt + 1) * 128, half, :], i_t[:]
                        )

    nc.compile()
    return nc


def _host_prep(x: np.ndarray, data: np.ndarray):
    """Per-core input maps: fp8 quantization, c2 fold rows, fp16 norm
    tables. All heavy FLOPs stay on device; host work is elementwise."""
    f8np = mybir.dt.np(mybir.dt.float8e4)

    xq = np.transpose(
        x.reshape(B, 2, 126, KSLOT, 8), (0, 3, 1, 2, 4)
    ).reshape(BK, D)

    # xq8 [128, KC, BK]: k_global = chunk*128 + partition; rows 0..D-1 carry
    # fp8(2*xq), rows D..D+NAUG-1 carry the aug coefficient 1, rest 0.
    xa = np.zeros((KC * 128, BK), dtype=f8np)
    xa[:D] = (xq.astype(np.float32) * 2.0).astype(np.float16).astype(f8np).T
    xa[D:D + NAUG] = np.float16(1.0)
    xq8 = np.ascontiguousarray(xa.reshape(KC, 128, BK).transpose(1, 0, 2))

    xs = xq.astype(np.float64).sum(axis=1)
    xs2 = np.ascontiguousarray(
        (2.0 * xs).astype(np.float16).reshape(NQT, 128).T
    )

    c = data.reshape(M, D)
    c64 = c.astype(np.float64)
    c2_all = np.einsum("md,md->m", c64, c64)
    cn2_all = D - 2.0 * c64.sum(axis=1) + c2_all

    # Greedy fp8 decomposition of c2 (rows <= 240; residual < 1e-2).
    resid = c2_all.copy()
    rows = []
    for _ in range(NAUG):
        r = np.minimum(resid, 240.0).astype(f8np).astype(np.float64)
        r = np.minimum(r, 240.0)
        rows.append(r)
        resid = resid - r

    in_maps = []
    for core in range(NCORES):
        s = core * MREAL
        e = min(s + MREAL, M)
        n = e - s
        ca = np.zeros((KC * 128, MLOC), dtype=f8np)
        ca[:D, :n] = c[s:e].astype(np.float16).astype(f8np).T
        for j in range(NAUG):
            aug = np.full(MLOC, -PAD_ROW, dtype=np.float64)
            aug[:n] = -rows[j][s:e]
            ca[D + j] = aug.astype(f8np)
        ct8 = np.ascontiguousarray(ca.reshape(KC, 128, MLOC).transpose(1, 0, 2))
        srep = np.full(MLOC, 60000.0, dtype=np.float16)
        srep[:n] = (c2_all[s:e] + cn2_all[s:e]).astype(np.float16)
        in_maps.append({
            "xq8": xq8,
            "ct8": ct8,
            "srep": np.ascontiguousarray(
                np.broadcast_to(srep[None, :], (128, MLOC))
            ),
            "xs2": xs2,
        })
    return in_maps


def _merge(results, x: np.ndarray, data: np.ndarray):
    """Exact f64 rescore of the 128 screened candidates per query, with the
    reference's [d0,d1,d1,d0]-first-occurrence tie-breaking."""
    # Candidate global rows: core*MREAL + half*1024 + idx, valid if < M.
    idx = np.stack([r["outi"].astype(np.int64) for r in results])  # [8,1024,2,8]
    half_off = np.arange(2, dtype=np.int64).reshape(1, 1, 2, 1) * 1024
    core_off = np.arange(NCORES, dtype=np.int64).reshape(NCORES, 1, 1, 1) * MREAL
    cand = (idx + half_off + core_off).transpose(1, 0, 2, 3).reshape(BK, -1)
    valid = cand < M
    cand_c = np.where(valid, cand, 0)

    xq = np.transpose(
        x.reshape(B, 2, 126, KSLOT, 8), (0, 3, 1, 2, 4)
    ).reshape(BK, D).astype(np.float64)
    xs = xq.sum(axis=1)
    c = data.reshape(M, D)

    NC = cand.shape[1]
    g = np.empty((BK, NC))
    side = np.empty((BK, NC), dtype=np.int64)
    blk = 128
    for i in range(0, BK, blk):
        cw = c[cand_c[i:i + blk]].astype(np.float64)       # [blk, NC, D]
        dot = np.einsum("qcd,qd->qc", cw, xq[i:i + blk])
        c2 = np.einsum("qcd,qcd->qc", cw, cw)
        cn2 = D - 2.0 * cw.sum(axis=2) + c2
        g0 = 2.0 * dot - c2                                # x2 - d0
        g1 = 2.0 * (xs[i:i + blk, None] - dot) - cn2       # x2 - d1
        g[i:i + blk] = np.maximum(g0, g1)
        side[i:i + blk] = (g1 > g0).astype(np.int64)       # d0 wins ties
    g[~valid] = -np.inf

    # Winner per query: max g; ties -> side 0 first, then lowest m.
    best = np.empty(BK, dtype=np.int64)
    for q in range(BK):
        order = np.lexsort((cand[q], side[q], -g[q]))
        best[q] = order[0]
    qq = np.arange(BK)
    return cand[qq, best] + side[qq, best] * M             # [1024]


def kernel(x: np.ndarray, data: np.ndarray) -> np.ndarray:
    if "nc" not in _compiled:
        _compiled["nc"] = _build_program()
    nc = _compiled["nc"]

    x = np.asarray(x)
    data = np.asarray(data)
    in_maps = _host_prep(x, data)
    res = run_bass_kernel_spmd(nc, in_maps, list(range(NCORES)))
    _compiled["last_result"] = res

    g = _merge(res.results, x, data).astype(np.int64)                # [1024]
    shifts = np.arange(NBITS, dtype=np.int64)
    bits = (g[:, None] >> shifts[None, :]) & 1
    return bits.astype(np.int32).reshape(B, KSLOT * NBITS)


# revision 9
# speedup vs baseline: 1.1106x; 1.1106x over previous
"""Trainium2 Bass kernel for nn_Encoder_79843442033106 (retrieval_knn).

Reference computation:
  queries xq[b,k,:] (1024 x 2016, fp16 values) are matched against a codebook
  c (16001 x 2016) under squared L2 distance, searching the concatenation
  [d0, d1, d1, d0] where d0 = ||x-c||^2 and d1 = ||x-(1-c)||^2; the argmin
  index is emitted LSB-first as 32 bits -> output [64, 512] int32.

Strategy (quantized screen + exact rescore, codebook axis M sharded 8 ways):
  * Device: one fp8e4 DoubleRow GEMM per core computes a screening score
      psum[q,m] ~= 2<x,c> - ||c||^2  (= x2 - d0, up to fp8 quantization)
    with K padded 2016->2048 (8 double-row contraction steps of 256) and
    -||c||^2 folded into 5 of the padding rows as a greedy fp8 decomposition
    (query-side coefficient 1).  fp8 DoubleRow streams 256 contraction rows
    per column-cycle -- 2x the fp16 matmul rate, measured ~61us/core for the
    1024x2048x2048 product vs ~133us for the exact fp16 equivalent.
  * DVE (fp16, 2x element rate): v = -t2 - (c2+cn2);  h = (v + 2xs) max t2
    so h[m] ~= max(x2-d0, x2-d1); then per 1024-wide half-chunk the DVE
    max/max_index ops emit the TOP-8 values and indices per query.
  * Host: 8 cores x 2 halves x top8 = 128 candidates/query; rescore them
    exactly in f64 with the reference's tie-breaking ((d, d0-before-d1,
    lowest m)); emit the argmin's 32 bits.  Screening noise is ~1.0 (std)
    while the true argmin ranks at worst 2nd within its half-chunk on this
    distribution -- top-8 gives 4x headroom (verified 0/1024 mismatches).
"""

import numpy as np

import concourse.bass as bass
import concourse.tile as tile
from concourse import bacc, mybir
from concourse.bass_utils import run_bass_kernel_spmd

# Problem constants (hardcoded per the harness contract).
B = 64
KSLOT = 16
D = 2016
M = 16001
NBITS = 32
BK = B * KSLOT           # 1024 queries
NCORES = 8
MLOC = 2048              # per-core codebook buffer columns (bank-aligned)
MREAL = 2001             # streamed columns per core (8*2001 >= 16001)
KC = 16                  # 128-row contraction chunks (16*128 = 2048 >= D+5)
NQT = BK // 128          # 8 query tiles
NAUG = 5                 # fp8 rows folding -c2 (greedy, cap 240)
PAD_ROW = np.float64(240.0)  # per-aug-row magnitude for padded columns
# PSUM-bank-aligned m-chunks: starts on 512-f32 boundaries, last one short.
CHUNKS = ((0, 512), (512, 1024), (1024, 1536), (1536, MREAL))
HALVES = ((0, 1024), (1024, MREAL))

_compiled = {}


def _build_program(repeat: int = 1) -> bass.Bass:
    """repeat>1 replays the whole compute body N times inside one NEFF --
    used only by the bench to measure per-iteration device time
    differentially (dispatch overhead cancels)."""
    f16 = mybir.dt.float16
    f32 = mybir.dt.float32
    f8 = mybir.dt.float8e4
    u32 = mybir.dt.uint32

    nc = bacc.Bacc("TRN2", debug=False, num_devices=NCORES)

    xq8 = nc.dram_tensor("xq8", [128, KC, BK], f8, kind="ExternalInput").ap()
    ct8 = nc.dram_tensor("ct8", [128, KC, MLOC], f8, kind="ExternalInput").ap()
    srep = nc.dram_tensor("srep", [128, MLOC], f16, kind="ExternalInput").ap()
    xs2 = nc.dram_tensor("xs2", [128, NQT], f16, kind="ExternalInput").ap()
    outv = nc.dram_tensor("outv", [BK, 2, 8], f16, kind="ExternalOutput").ap()
    outi = nc.dram_tensor("outi", [BK, 2, 8], u32, kind="ExternalOutput").ap()

    with tile.TileContext(nc) as tc:
        with (
            tc.tile_pool(name="const", bufs=1) as const_pool,
            tc.tile_pool(name="psum", bufs=2, space="PSUM") as psum_pool,
            tc.tile_pool(name="work", bufs=3) as work_pool,
            tc.tile_pool(name="outs", bufs=3) as out_pool,
        ):
            xs2_t = const_pool.tile([128, NQT], f16, tag="xs2")
            srep_t = const_pool.tile([128, MLOC], f16, tag="srep")
            srepn_t = const_pool.tile([128, MLOC], f16, tag="srepn")
            xt = const_pool.tile([128, KC, BK], f8, tag="xq8")
            ctt = const_pool.tile([128, KC, MLOC], f8, tag="ct8")
            # Queries first (small), then the codes the first matmuls touch.
            nc.sync.dma_start(xt[:], xq8[:, :, :])
            nc.sync.dma_start(xs2_t[:], xs2[:, :])
            nc.sync.dma_start(ctt[:], ct8[:, :, :])
            nc.sync.dma_start(srep_t[:], srep[:, :])
            nc.scalar.mul(srepn_t[:], srep_t[:], -1.0)

            for rep in range(repeat):
                for qt in range(NQT):
                    # GEMM: psum[q, m] = 2<x,c> - c2 (fp8 DoubleRow, f32 acc).
                    # Chunks start on PSUM bank boundaries (512 f32): a
                    # matmul output may not straddle banks.
                    ps = psum_pool.tile([128, MLOC], f32, tag="ps")
                    for j in range(KC // 2):
                        for lo, hi in CHUNKS:
                            nc.tensor.matmul(
                                ps[:, lo:hi],
                                lhsT=xt[:, 2 * j:2 * j + 2,
                                        qt * 128:(qt + 1) * 128],
                                rhs=ctt[:, 2 * j:2 * j + 2, lo:hi],
                                start=(j == 0),
                                stop=(j == KC // 2 - 1),
                                perf_mode=mybir.MatmulPerfMode.DoubleRow,
                            )

                    # ACT stages PSUM->SBUF as fp16 (frees the PSUM banks and
                    # gives the DVE its 2x 16-bit element rate); per-chunk so
                    # the copy starts as soon as each bank's accumulation ends.
                    t2 = work_pool.tile([128, MLOC], f16, tag="t2")
                    for lo, hi in CHUNKS:
                        nc.scalar.copy(t2[:, lo:hi], ps[:, lo:hi])
                    v = work_pool.tile([128, MLOC], f16, tag="v")
                    h = work_pool.tile([128, MLOC], f16, tag="h")
                    for half, (hlo, hhi) in enumerate(HALVES):
                        sl = slice(hlo, hhi)
                        # v on the Pool engine: the kernel is DVE-bound (the
                        # top-8 scans must run there), so the first
                        # elementwise op moves to the otherwise-idle Pool
                        # engine as a plain tensor_tensor subtract.
                        nc.gpsimd.tensor_tensor(
                            v[:, sl], in0=srepn_t[:, sl], in1=t2[:, sl],
                            op=mybir.AluOpType.subtract,
                        )
                        nc.vector.scalar_tensor_tensor(
                            h[:, sl], in0=v[:, sl],
                            scalar=xs2_t[:, qt:qt + 1], in1=t2[:, sl],
                            op0=mybir.AluOpType.add, op1=mybir.AluOpType.max,
                        )
                        v_t = out_pool.tile([128, 8], f16, tag="v8")
                        nc.vector.max(v_t[:], h[:, sl])
                        i_t = out_pool.tile([128, 8], u32, tag="i8")
                        nc.vector.max_index(i_t[:], v_t[:], h[:, sl])
                        nc.sync.dma_start(
                            outv[qt * 128:(qt + 1) * 128, half, :], v_t[:]
                        )
                        nc.sync.dma_start(
                            outi[qt * 128:(qt + 1) * 128, half, :], i_t[:]
                        )

    nc.compile()
    return nc


def _host_prep(x: np.ndarray, data: np.ndarray):
    """Per-core input maps: fp8 quantization, c2 fold rows, fp16 norm
    tables. All heavy FLOPs stay on device; host work is elementwise."""
    f8np = mybir.dt.np(mybir.dt.float8e4)

    xq = np.transpose(
        x.reshape(B, 2, 126, KSLOT, 8), (0, 3, 1, 2, 4)
    ).reshape(BK, D)

    # xq8 [128, KC, BK]: k_global = chunk*128 + partition; rows 0..D-1 carry
    # fp8(2*xq), rows D..D+NAUG-1 carry the aug coefficient 1, rest 0.
    xa = np.zeros((KC * 128, BK), dtype=f8np)
    xa[:D] = (xq.astype(np.float32) * 2.0).astype(np.float16).astype(f8np).T
    xa[D:D + NAUG] = np.float16(1.0)
    xq8 = np.ascontiguousarray(xa.reshape(KC, 128, BK).transpose(1, 0, 2))

    xs = xq.astype(np.float64).sum(axis=1)
    xs2 = np.ascontiguousarray(
        (2.0 * xs).astype(np.float16).reshape(NQT, 128).T
    )

    c = data.reshape(M, D)
    c64 = c.astype(np.float64)
    c2_all = np.einsum("md,md->m", c64, c64)
    cn2_all = D - 2.0 * c64.sum(axis=1) + c2_all

    # Greedy fp8 decomposition of c2 (rows <= 240; residual < 1e-2).
    resid = c2_all.copy()
    rows = []
    for _ in range(NAUG):
        r = np.minimum(resid, 240.0).astype(f8np).astype(np.float64)
        r = np.minimum(r, 240.0)
        rows.append(r)
        resid = resid - r

    in_maps = []
    for core in range(NCORES):
        s = core * MREAL
        e = min(s + MREAL, M)
        n = e - s
        ca = np.zeros((KC * 128, MLOC), dtype=f8np)
        ca[:D, :n] = c[s:e].astype(np.float16).astype(f8np).T
        for j in range(NAUG):
            aug = np.full(MLOC, -PAD_ROW, dtype=np.float64)
            aug[:n] = -rows[j][s:e]
            ca[D + j] = aug.astype(f8np)
        ct8 = np.ascontiguousarray(ca.reshape(KC, 128, MLOC).transpose(1, 0, 2))
        srep = np.full(MLOC, 60000.0, dtype=np.float16)
        srep[:n] = (c2_all[s:e] + cn2_all[s:e]).astype(np.float16)
        in_maps.append({
            "xq8": xq8,
            "ct8": ct8,
            "srep": np.ascontiguousarray(
                np.broadcast_to(srep[None, :], (128, MLOC))
            ),
            "xs2": xs2,
        })
    return in_maps


def _merge(results, x: np.ndarray, data: np.ndarray):
    """Exact f64 rescore of the 128 screened candidates per query, with the
    reference's [d0,d1,d1,d0]-first-occurrence tie-breaking."""
    # Candidate global rows: core*MREAL + half*1024 + idx, valid if < M.
    idx = np.stack([r["outi"].astype(np.int64) for r in results])  # [8,1024,2,8]
    half_off = np.arange(2, dtype=np.int64).reshape(1, 1, 2, 1) * 1024
    core_off = np.arange(NCORES, dtype=np.int64).reshape(NCORES, 1, 1, 1) * MREAL
    cand = (idx + half_off + core_off).transpose(1, 0, 2, 3).reshape(BK, -1)
    valid = cand < M
    cand_c = np.where(valid, cand, 0)

    xq = np.transpose(
        x.reshape(B, 2, 126, KSLOT, 8), (0, 3, 1, 2, 4)
    ).reshape(BK, D).astype(np.float64)
    xs = xq.sum(axis=1)
    c = data.reshape(M, D)

    NC = cand.shape[1]
    g = np.empty((BK, NC))
    side = np.empty((BK, NC), dtype=np.int64)
    blk = 128
    for i in range(0, BK, blk):
        cw = c[cand_c[i:i + blk]].astype(np.float64)       # [blk, NC, D]
        dot = np.einsum("qcd,qd->qc", cw, xq[i:i + blk])
        c2 = np.einsum("qcd,qcd->qc", cw, cw)
        cn2 = D - 2.0 * cw.sum(axis=2) + c2
        g0 = 2.0 * dot - c2                                # x2 - d0
        g1 = 2.0 * (xs[i:i + blk, None] - dot) - cn2       # x2 - d1
        g[i:i + blk] = np.maximum(g0, g1)
        side[i:i + blk] = (g1 > g0).astype(np.int64)       # d0 wins ties
    g[~valid] = -np.inf

    # Winner per query: max g; ties -> side 0 first, then lowest m.
    best = np.empty(BK, dtype=np.int64)
    for q in range(BK):
        order = np.lexsort((cand[q], side[q], -g[q]))
        best[q] = order[0]
    qq = np.arange(BK)
    return cand[qq, best] + side[qq, best] * M             # [1024]


def kernel(x: np.ndarray, data: np.ndarray) -> np.ndarray:
    if "nc" not in _compiled:
        _compiled["nc"] = _build_program()
    nc = _compiled["nc"]

    x = np.asarray(x)
    data = np.asarray(data)
    in_maps = _host_prep(x, data)
    res = run_bass_kernel_spmd(nc, in_maps, list(range(NCORES)))
    _compiled["last_result"] = res

    g = _merge(res.results, x, data).astype(np.int64)                # [1024]
    shifts = np.arange(NBITS, dtype=np.int64)
    bits = (g[:, None] >> shifts[None, :]) & 1
    return bits.astype(np.int32).reshape(B, KSLOT * NBITS)
